# revision 2
# baseline (speedup 1.0000x reference)
"""Causal self-attention (B=2, T=2048, D=1024, H=16, Dh=64) on 8 TRN2 cores.

Sharding: core c -> batch b = c//4 (data parallel), head group g = c%4
(tensor parallel, 4 heads = 256 dims). Each core computes a full-shape
[T, D] bf16 partial of the output projection for its (b, g); the host
sums the 4 head-group partials per batch in f32.

Fully fused chunk-major pipeline (chunk n = 512 query cols = region n):
for each chunk: q/k/v projections of chunk n+1 and out-proj of chunk
n-1 are emitted as PE "filler" work interleaved into the attention
j-stream of chunk n, so the ScalarE exp stream (the phase-2 bottleneck,
~75us) hides under PE work and the PE never idles waiting on exp.
Causal diag masking is done on the PE itself (identity-stationary
matmul accumulating a lower-triangular -3e4 constant into S.T before
exp) instead of VectorE multiplies, removing the Scalar->Vector->PE
dependency hop. All PSUM rotates through one shared [128,1024] slot
tag (2 bufs) + the O.T accumulators (2x2 bufs), exactly 8 banks.
"""

import numpy as np
from collections import deque
from contextlib import ExitStack

import concourse.bass as bass
import concourse.tile as tile
from concourse import bacc, mybir
from concourse.bass_utils import run_bass_kernel_spmd

F32 = mybir.dt.float32
BF16 = mybir.dt.bfloat16
CDT = BF16

B, T, D = 2, 2048, 1024
H_TOT, DH = 16, 64
HL = 4                # local heads per core
DG = HL * DH          # 256 local head dims
NT = T // 128         # 16 t-tiles
NCH = T // 512        # 4 t-chunks
CT = D // 128         # 8 c-tiles

_CACHE = {}


def build():
    nc = bacc.Bacc("TRN2", target_bir_lowering=False, debug=False, num_devices=8)
    xT_d = nc.dram_tensor("xT", [NCH, 128, CT, 512], CDT, kind="ExternalInput").ap()
    wq_d = nc.dram_tensor("wq", [128, CT, DG], CDT, kind="ExternalInput").ap()
    wk_d = nc.dram_tensor("wk", [128, CT, DG], CDT, kind="ExternalInput").ap()
    wv_d = nc.dram_tensor("wv", [128, CT, DG], CDT, kind="ExternalInput").ap()
    wo_d = nc.dram_tensor("wo", [128, 2, D], CDT, kind="ExternalInput").ap()
    tri_d = nc.dram_tensor("tri", [128, 128], CDT, kind="ExternalInput").ap()
    idn_d = nc.dram_tensor("idn", [128, 128], CDT, kind="ExternalInput").ap()
    out_d = nc.dram_tensor("out", [T, D], CDT, kind="ExternalOutput").ap()

    with tile.TileContext(nc) as tc:
        with ExitStack() as ctx:
            cons = ctx.enter_context(tc.tile_pool(name="cons", bufs=1))
            xp = ctx.enter_context(tc.tile_pool(name="xp", bufs=2))
            cp = ctx.enter_context(tc.tile_pool(name="cp", bufs=3))
            pp = ctx.enter_context(tc.tile_pool(name="pp", bufs=4))
            outp = ctx.enter_context(tc.tile_pool(name="outp", bufs=4))
            ps = ctx.enter_context(tc.tile_pool(name="ps", bufs=2, space="PSUM"))
            opool = ctx.enter_context(
                tc.tile_pool(name="opool", bufs=2, space="PSUM")
            )

            wq_sb = cons.tile([128, CT, DG], CDT)
            wk_sb = cons.tile([128, CT, DG], CDT)
            wv_sb = cons.tile([128, CT, DG], CDT)
            wo_sb = cons.tile([128, 2, D], CDT)
            tri_sb = cons.tile([128, 128], CDT)
            idn_sb = cons.tile([128, 128], CDT)

            qsb = cons.tile([128, 2, T], CDT)
            ksb = cons.tile([128, 2, T], CDT)
            lrows = cons.tile([128, T], F32)
            v_sb = cons.tile([128, NT, HL, DH + 1], CDT)
            nc.vector.memset(v_sb[:, :, :, DH], 1.0)
            y_sb = cons.tile([128, 2, T], CDT)

            # ---- input DMA, priority order: wq + x0 first, wk next (all
            # needed within ~5us), then prefetch x1 / wv / consts / wo ----
            x_tiles = [None] * NCH

            def dma_x(n, eng_lo, eng_hi):
                x_tiles[n] = xp.tile([128, CT, 512], CDT, tag="x", name=f"x{n}")
                eng_lo.dma_start(x_tiles[n][:, 0:4, :], xT_d[n, :, 0:4, :])
                eng_hi.dma_start(x_tiles[n][:, 4:CT, :], xT_d[n, :, 4:CT, :])

            nc.sync.dma_start(wq_sb[:], wq_d[:])
            dma_x(0, nc.scalar, nc.gpsimd)
            nc.sync.dma_start(wk_sb[:], wk_d[:])
            nc.scalar.dma_start(wv_sb[:], wv_d[:])
            dma_x(1, nc.sync, nc.gpsimd)
            nc.scalar.dma_start(tri_sb[:], tri_d[:])
            nc.scalar.dma_start(idn_sb[:], idn_d[:])
            nc.gpsimd.dma_start(wo_sb[:], wo_d[:])

            # ---- PE filler closures: projections of chunk n (q, k, v) and
            # out-projection of earlier chunks, drained into the attention
            # j-stream so the PE stays busy while ScalarE chews exp ----
            def proj_qk_closures(n, w_sb, dst):
                cls = []
                slot = {}

                def mm(j2, ct):
                    def run():
                        if "t" not in slot:
                            slot["t"] = ps.tile(
                                [128, 1024], F32, tag="ps", name=f"qk{n}"
                            )
                        nc.tensor.matmul(
                            slot["t"][:, 512 * j2 : 512 * (j2 + 1)],
                            w_sb[:, ct, 128 * j2 : 128 * (j2 + 1)],
                            x_tiles[n][:, ct, :],
                            start=(ct == 0),
                            stop=(ct == CT - 1),
                            skip_group_check=True,
                        )

                    return run

                for j2 in range(2):
                    for ct in range(CT):
                        cls.append(mm(j2, ct))

                def cast():
                    nc.vector.tensor_copy(
                        dst[:, :, 512 * n : 512 * (n + 1)],
                        slot["t"][:].rearrange("p (g c) -> p g c", g=2),
                    )

                cls.append(cast)
                return cls

            def proj_v_closures(n):
                cls = []
                slot = {}

                def mm(i, ct):
                    def run():
                        if "t" not in slot:
                            slot["t"] = ps.tile(
                                [128, 1024], F32, tag="ps", name=f"v{n}"
                            )
                        nc.tensor.matmul(
                            slot["t"][:, 256 * i : 256 * (i + 1)],
                            x_tiles[n][:, ct, 128 * i : 128 * (i + 1)],
                            wv_sb[:, ct, :],
                            start=(ct == 0),
                            stop=(ct == CT - 1),
                            skip_group_check=True,
                        )

                    return run

                for i in range(4):
                    for ct in range(CT):
                        cls.append(mm(i, ct))

                def cast():
                    nc.vector.tensor_copy(
                        v_sb[:, 4 * n : 4 * (n + 1), :, 0:DH],
                        slot["t"][:].rearrange(
                            "p (i h d) -> p i h d", i=4, h=HL
                        ),
                    )

                cls.append(cast)
                return cls

            def p3_closures(n):
                # out[t,:] for t-tiles of chunk n; both oc halves in one slot
                cls = []
                for i in range(4 * n, 4 * n + 4):

                    def run(i=i):
                        po = ps.tile([128, 1024], F32, tag="ps", name=f"po{i}")
                        for oc in range(2):
                            for g2 in range(2):
                                nc.tensor.matmul(
                                    po[:, 512 * oc : 512 * (oc + 1)],
                                    y_sb[:, g2, 128 * i : 128 * (i + 1)],
                                    wo_sb[:, g2, 512 * oc : 512 * (oc + 1)],
                                    start=(g2 == 0),
                                    stop=(g2 == 1),
                                    skip_group_check=True,
                                )
                        o_sb = outp.tile([128, 1024], CDT, tag="o")
                        if i % 2 == 0:
                            nc.vector.tensor_copy(o_sb[:], po[:])
                        else:
                            nc.scalar.copy(o_sb[:], po[:])
                        eng = (nc.gpsimd, nc.sync, nc.scalar)[i % 3]
                        eng.dma_start(
                            out_d[128 * i : 128 * (i + 1), :], o_sb[:]
                        )

                    cls.append(run)
                return cls

            # ---- attention for (pair p, region n) with filler draining ----
            def attention(p, n, fillers, pops):
                c0r, c1r = 512 * n, 512 * (n + 1)
                jlast = 4 * n + 3
                oTa = opool.tile([DH + 1, 512], F32, tag="oTa", name=f"oTa{p}_{n}")
                oTb = opool.tile([DH + 1, 512], F32, tag="oTb", name=f"oTb{p}_{n}")

                def emit_st(j):
                    c0 = max(c0r, 128 * j)
                    w = c1r - c0
                    masked = j >= 4 * n
                    sT = ps.tile([128, 1024], F32, tag="ps", name=f"sT{p}_{n}_{j}")
                    nc.tensor.matmul(
                        sT[:, 0:w],
                        ksb[0:DH, p, 128 * j : 128 * (j + 1)],
                        qsb[0:DH, p, c0:c1r],
                        start=True,
                        stop=not masked,
                        skip_group_check=True,
                    )
                    nc.tensor.matmul(
                        sT[:, 512 : 512 + w],
                        ksb[DH:128, p, 128 * j : 128 * (j + 1)],
                        qsb[DH:128, p, c0:c1r],
                        start=True,
                        stop=not masked,
                        skip_group_check=True,
                    )
                    if masked:  # diag block at rel cols [0,128): add -3e4 above
                        nc.tensor.matmul(
                            sT[:, 0:128], idn_sb[:], tri_sb[:],
                            start=False, stop=True, skip_group_check=True,
                        )
                        nc.tensor.matmul(
                            sT[:, 512:640], idn_sb[:], tri_sb[:],
                            start=False, stop=True, skip_group_check=True,
                        )
                    pT = pp.tile([128, 1024], CDT, tag="pT", name=f"pT{p}_{n}_{j}")
                    nc.scalar.activation(
                        pT[:, 0 : 512 + w],
                        sT[:, 0 : 512 + w],
                        mybir.ActivationFunctionType.Exp,
                        scale=0.125,
                    )
                    return pT

                def emit_pv(j, pT):
                    c0 = max(c0r, 128 * j)
                    w = c1r - c0
                    nc.tensor.matmul(
                        oTa[:, c0 - c0r :],
                        v_sb[:, j, 2 * p, :],
                        pT[:, 0:w],
                        start=(j == 0),
                        stop=(j == jlast),
                        skip_group_check=True,
                    )
                    nc.tensor.matmul(
                        oTb[:, c0 - c0r :],
                        v_sb[:, j, 2 * p + 1, :],
                        pT[:, 512 : 512 + w],
                        start=(j == 0),
                        stop=(j == jlast),
                        skip_group_check=True,
                    )

                prev = None
                for j in range(jlast + 1):
                    for _ in range(pops):
                        if fillers:
                            fillers.popleft()()
                    pT = emit_st(j)
                    if prev is not None:
                        emit_pv(*prev)
                    prev = (j, pT)
                emit_pv(*prev)

                # softmax normalization for both heads of this region
                lt_sb = cp.tile([128, T // 128], F32, tag="lt", name=f"lt{p}_{n}")
                rt_sb = cp.tile([128, T // 128], F32, tag="rt", name=f"rt{p}_{n}")
                r_sb = cp.tile([1, T], F32, tag="r", name=f"r{p}_{n}")
                rb_sb = cp.tile([128, T], F32, tag="rb", name=f"rb{p}_{n}")
                for h, oT in ((2 * p, oTa), (2 * p + 1, oTb)):
                    hp = 64 * (h % 2)
                    nc.vector.tensor_copy(
                        y_sb[hp : hp + DH, p, c0r:c1r], oT[0:DH, :]
                    )
                    nc.vector.tensor_copy(
                        lrows[32 * h : 32 * h + 1, c0r:c1r], oT[DH : DH + 1, :]
                    )
                    nc.sync.dma_start(
                        lt_sb[32 * n : 32 * (n + 1), :],
                        lrows[32 * h : 32 * h + 1, c0r:c1r],
                    )
                    nc.vector.reciprocal(
                        rt_sb[32 * n : 32 * (n + 1), :],
                        lt_sb[32 * n : 32 * (n + 1), :],
                    )
                    nc.sync.dma_start(
                        r_sb[:, c0r:c1r], rt_sb[32 * n : 32 * (n + 1), :]
                    )
                    nc.gpsimd.partition_broadcast(
                        rb_sb[:, c0r:c1r], r_sb[:, c0r:c1r]
                    )
                    nc.vector.tensor_mul(
                        y_sb[hp : hp + DH, p, c0r:c1r],
                        y_sb[hp : hp + DH, p, c0r:c1r],
                        rb_sb[hp : hp + DH, c0r:c1r],
                    )

            # ---- main fused loop ----
            # chunk 0 projections run up front (ScalarE has nothing yet)
            for c in proj_qk_closures(0, wq_sb, qsb):
                c()
            for c in proj_qk_closures(0, wk_sb, ksb):
                c()
            for c in proj_v_closures(0):
                c()

            fillers = deque()
            for n in range(NCH):
                if n < NCH - 1:
                    fillers.extend(proj_qk_closures(n + 1, wq_sb, qsb))
                    fillers.extend(proj_qk_closures(n + 1, wk_sb, ksb))
                    fillers.extend(proj_v_closures(n + 1))
                if n == 1:

                    def dma_x2():
                        dma_x(2, nc.sync, nc.scalar)

                    fillers.appendleft(dma_x2)
                if n == 2:

                    def dma_x3():
                        dma_x(3, nc.sync, nc.scalar)

                    fillers.appendleft(dma_x3)
                n_j = 2 * (4 * n + 4)
                pops = max(1, -(-len(fillers) // n_j))
                attention(0, n, fillers, pops)
                attention(1, n, fillers, pops)
                while fillers:
                    fillers.popleft()()
                fillers.extend(p3_closures(n))
            while fillers:
                fillers.popleft()()
    nc.compile()
    return nc


def make_in_maps(x, Wq, Wk, Wv, Wo):
    import ml_dtypes

    cnp = ml_dtypes.bfloat16
    r = np.arange(128)
    tri = (-30000.0 * (r[:, None] > r[None, :])).astype(cnp)  # [tk, tq]
    idn = np.eye(128, dtype=cnp)
    in_maps = []
    for c in range(8):
        b, g = c // 4, c % 4
        rows = slice(DG * g, DG * (g + 1))
        in_maps.append(
            {
                "xT": np.ascontiguousarray(
                    x[b].T.reshape(CT, 128, NCH, 512).transpose(2, 1, 0, 3)
                ).astype(cnp),
                "wq": np.ascontiguousarray(
                    Wq[rows].T.reshape(CT, 128, DG).transpose(1, 0, 2)
                ).astype(cnp),
                "wk": np.ascontiguousarray(
                    Wk[rows].T.reshape(CT, 128, DG).transpose(1, 0, 2)
                ).astype(cnp),
                "wv": np.ascontiguousarray(
                    Wv[rows].T.reshape(CT, 128, DG).transpose(1, 0, 2)
                ).astype(cnp),
                "wo": np.ascontiguousarray(
                    Wo[:, rows].T.reshape(2, 128, D).transpose(1, 0, 2)
                ).astype(cnp),
                "tri": tri,
                "idn": idn,
            }
        )
    return in_maps


def _run(x, Wq, Wk, Wv, Wo, trace=False):
    if "nc" not in _CACHE:
        _CACHE["nc"] = build()
    nc = _CACHE["nc"]
    in_maps = make_in_maps(x, Wq, Wk, Wv, Wo)
    res = run_bass_kernel_spmd(nc, in_maps, core_ids=list(range(8)), trace=trace)
    out = np.zeros((B, T, D), dtype=np.float32)
    for c in range(8):
        out[c // 4] += np.asarray(res.results[c]["out"], dtype=np.float32)
    return out, res


def kernel(x, Wq, Wk, Wv, Wo):
    out, _ = _run(
        np.asarray(x, dtype=np.float32),
        np.asarray(Wq, dtype=np.float32),
        np.asarray(Wk, dtype=np.float32),
        np.asarray(Wv, dtype=np.float32),
        np.asarray(Wo, dtype=np.float32),
    )
    return out


# revision 9
# speedup vs baseline: 1.1305x; 1.1305x over previous
"""Causal self-attention (B=2, T=2048, D=1024, H=16, Dh=64) on 8 TRN2 cores.

Sharding: core c -> batch b = c//4 (data parallel), head group g = c%4
(tensor parallel, 4 heads = 256 dims). Each core computes a full-shape
[T, D] bf16 partial of the output projection for its (b, g); the host
sums the 4 head-group partials per batch in f32.

Fully fused chunk-major pipeline (chunk n = 512 query cols = region n):
for each chunk: q/k/v projections of chunk n+1 and out-proj of chunk
n-1 are emitted as PE "filler" work interleaved into the attention
j-stream of chunk n, so the ScalarE exp stream (the phase-2 bottleneck,
~75us) hides under PE work and the PE never idles waiting on exp.
Causal diag masking is done on the PE itself (identity-stationary
matmul accumulating a lower-triangular -3e4 constant into S.T before
exp) instead of VectorE multiplies, removing the Scalar->Vector->PE
dependency hop. All PSUM rotates through one shared [128,1024] slot
tag (2 bufs) + the O.T accumulators (2x2 bufs), exactly 8 banks.
"""

import numpy as np
from collections import deque
from contextlib import ExitStack

import concourse.bass as bass
import concourse.tile as tile
from concourse import bacc, mybir
from concourse.bass_utils import run_bass_kernel_spmd

F32 = mybir.dt.float32
BF16 = mybir.dt.bfloat16
CDT = BF16

B, T, D = 2, 2048, 1024
H_TOT, DH = 16, 64
HL = 4                # local heads per core
DG = HL * DH          # 256 local head dims
NT = T // 128         # 16 t-tiles
NCH = T // 512        # 4 t-chunks
CT = D // 128         # 8 c-tiles

_CACHE = {}


def build():
    nc = bacc.Bacc("TRN2", target_bir_lowering=False, debug=False, num_devices=8)
    xT_d = nc.dram_tensor("xT", [NCH, 128, CT, 512], CDT, kind="ExternalInput").ap()
    wq_d = nc.dram_tensor("wq", [128, CT, DG], CDT, kind="ExternalInput").ap()
    wk_d = nc.dram_tensor("wk", [128, CT, DG], CDT, kind="ExternalInput").ap()
    wv_d = nc.dram_tensor("wv", [128, CT, DG], CDT, kind="ExternalInput").ap()
    wo_d = nc.dram_tensor("wo", [128, 2, D], CDT, kind="ExternalInput").ap()
    tri_d = nc.dram_tensor("tri", [128, 128], CDT, kind="ExternalInput").ap()
    idn_d = nc.dram_tensor("idn", [128, 128], CDT, kind="ExternalInput").ap()
    out_d = nc.dram_tensor("out", [T, D], CDT, kind="ExternalOutput").ap()

    with tile.TileContext(nc) as tc:
        with ExitStack() as ctx:
            cons = ctx.enter_context(tc.tile_pool(name="cons", bufs=1))
            xp = ctx.enter_context(tc.tile_pool(name="xp", bufs=2))
            cp = ctx.enter_context(tc.tile_pool(name="cp", bufs=3))
            pp = ctx.enter_context(tc.tile_pool(name="pp", bufs=4))
            outp = ctx.enter_context(tc.tile_pool(name="outp", bufs=4))
            ps = ctx.enter_context(tc.tile_pool(name="ps", bufs=2, space="PSUM"))
            opool = ctx.enter_context(
                tc.tile_pool(name="opool", bufs=2, space="PSUM")
            )

            wq_sb = cons.tile([128, CT, DG], CDT)
            wk_sb = cons.tile([128, CT, DG], CDT)
            wv_sb = cons.tile([128, CT, DG], CDT)
            wo_sb = cons.tile([128, 2, D], CDT)
            tri_sb = cons.tile([128, 128], CDT)
            idn_sb = cons.tile([128, 128], CDT)

            # warm the ACT exp table during the initial DMA wait
            warm = cons.tile([1, 8], F32)
            nc.scalar.activation(
                warm[:], warm[:], mybir.ActivationFunctionType.Exp
            )

            qsb = cons.tile([128, 2, T], CDT)
            ksb = cons.tile([128, 2, T], CDT)
            lrows = cons.tile([128, T], F32)
            v_sb = cons.tile([128, NT, HL, DH + 1], CDT)
            nc.vector.memset(v_sb[:, :, :, DH], 1.0)
            y_sb = cons.tile([128, 2, T], CDT)

            # ---- input DMA, priority order: wq + x0 first, wk next (all
            # needed within ~5us), then prefetch x1 / wv / consts / wo ----
            x_tiles = [None] * NCH

            def dma_x(n, eng_lo, eng_hi):
                x_tiles[n] = xp.tile([128, CT, 512], CDT, tag="x", name=f"x{n}")
                eng_lo.dma_start(x_tiles[n][:, 0:4, :], xT_d[n, :, 0:4, :])
                eng_hi.dma_start(x_tiles[n][:, 4:CT, :], xT_d[n, :, 4:CT, :])

            nc.sync.dma_start(wq_sb[:], wq_d[:])
            dma_x(0, nc.scalar, nc.gpsimd)
            nc.sync.dma_start(wk_sb[:], wk_d[:])
            nc.sync.dma_start(tri_sb[:], tri_d[:])
            nc.sync.dma_start(idn_sb[:], idn_d[:])
            nc.scalar.dma_start(wv_sb[:], wv_d[:])
            dma_x(1, nc.sync, nc.gpsimd)
            nc.gpsimd.dma_start(wo_sb[:], wo_d[:])

            # ---- PE filler closures: projections of chunk n (q, k, v) and
            # out-projection of earlier chunks, drained into the attention
            # j-stream so the PE stays busy while ScalarE chews exp ----
            def proj_qk(n, w_sb, dst, j2):
                # one self-contained filler: 8 accumulating matmuls + cast
                # for one pair half (j2) of the q/k projection of chunk n
                def run():
                    slot = ps.tile([128, 1024], F32, tag="ps", name=f"qk{n}_{j2}")
                    for ct in range(CT):
                        nc.tensor.matmul(
                            slot[:, 0:512],
                            w_sb[:, ct, 128 * j2 : 128 * (j2 + 1)],
                            x_tiles[n][:, ct, :],
                            start=(ct == 0),
                            stop=(ct == CT - 1),
                            skip_group_check=True,
                        )
                    nc.vector.tensor_copy(
                        dst[:, j2, 512 * n : 512 * (n + 1)], slot[:, 0:512]
                    )

                return run

            def proj_v(n, h2):
                # v projection of chunk n for t-subtiles {2*h2, 2*h2+1}
                def run():
                    slot = ps.tile([128, 1024], F32, tag="ps", name=f"v{n}_{h2}")
                    for i in (2 * h2, 2 * h2 + 1):
                        for ct in range(CT):
                            nc.tensor.matmul(
                                slot[:, 256 * (i % 2) : 256 * (i % 2 + 1)],
                                x_tiles[n][:, ct, 128 * i : 128 * (i + 1)],
                                wv_sb[:, ct, :],
                                start=(ct == 0),
                                stop=(ct == CT - 1),
                                skip_group_check=True,
                            )
                    ti = 4 * n + 2 * h2
                    nc.vector.tensor_copy(
                        v_sb[:, ti : ti + 2, :, 0:DH],
                        slot[:, 0:512].rearrange(
                            "p (i h d) -> p i h d", i=2, h=HL
                        ),
                    )

                return run

            def p3_closures(n):
                # out[t,:] for t-tiles of chunk n; both oc halves in one slot
                cls = []
                for i in range(4 * n, 4 * n + 4):

                    def run(i=i):
                        po = ps.tile([128, 1024], F32, tag="ps", name=f"po{i}")
                        for oc in range(2):
                            for g2 in range(2):
                                nc.tensor.matmul(
                                    po[:, 512 * oc : 512 * (oc + 1)],
                                    y_sb[:, g2, 128 * i : 128 * (i + 1)],
                                    wo_sb[:, g2, 512 * oc : 512 * (oc + 1)],
                                    start=(g2 == 0),
                                    stop=(g2 == 1),
                                    skip_group_check=True,
                                )
                        o_sb = outp.tile([128, 1024], CDT, tag="o")
                        nc.vector.tensor_copy(o_sb[:], po[:])
                        eng = (nc.sync, nc.scalar)[i % 2]
                        eng.dma_start(
                            out_d[128 * i : 128 * (i + 1), :], o_sb[:]
                        )

                    cls.append(run)
                return cls

            # ---- attention for (pair p, region n) with filler draining ----
            def attention(p, n, fillers, pops):
                c0r, c1r = 512 * n, 512 * (n + 1)
                jlast = 4 * n + 3
                oTa = opool.tile([DH + 1, 512], F32, tag="oTa", name=f"oTa{p}_{n}")
                oTb = opool.tile([DH + 1, 512], F32, tag="oTb", name=f"oTb{p}_{n}")

                def emit_st(j):
                    c0 = max(c0r, 128 * j)
                    w = c1r - c0
                    masked = j >= 4 * n
                    sT = ps.tile([128, 1024], F32, tag="ps", name=f"sT{p}_{n}_{j}")
                    nc.tensor.matmul(
                        sT[:, 0:w],
                        ksb[0:DH, p, 128 * j : 128 * (j + 1)],
                        qsb[0:DH, p, c0:c1r],
                        start=True,
                        stop=not masked,
                        skip_group_check=True,
                    )
                    nc.tensor.matmul(
                        sT[:, 512 : 512 + w],
                        ksb[DH:128, p, 128 * j : 128 * (j + 1)],
                        qsb[DH:128, p, c0:c1r],
                        start=True,
                        stop=not masked,
                        skip_group_check=True,
                    )
                    if masked:  # diag block at rel cols [0,128): add -3e4 above
                        nc.tensor.matmul(
                            sT[:, 0:128], idn_sb[:], tri_sb[:],
                            start=False, stop=True, skip_group_check=True,
                        )
                        nc.tensor.matmul(
                            sT[:, 512:640], idn_sb[:], tri_sb[:],
                            start=False, stop=True, skip_group_check=True,
                        )
                    pT = pp.tile([128, 1024], CDT, tag="pT", name=f"pT{p}_{n}_{j}")
                    nc.scalar.activation(
                        pT[:, 0 : 512 + w],
                        sT[:, 0 : 512 + w],
                        mybir.ActivationFunctionType.Exp,
                        scale=0.125,
                    )
                    return pT

                def emit_pv(j, pT):
                    c0 = max(c0r, 128 * j)
                    w = c1r - c0
                    nc.tensor.matmul(
                        oTa[:, c0 - c0r :],
                        v_sb[:, j, 2 * p, :],
                        pT[:, 0:w],
                        start=(j == 0),
                        stop=(j == jlast),
                        skip_group_check=True,
                    )
                    nc.tensor.matmul(
                        oTb[:, c0 - c0r :],
                        v_sb[:, j, 2 * p + 1, :],
                        pT[:, 512 : 512 + w],
                        start=(j == 0),
                        stop=(j == jlast),
                        skip_group_check=True,
                    )

                prev = None
                for j in range(jlast + 1):
                    for _ in range(pops):
                        if fillers:
                            fillers.popleft()()
                    pT = emit_st(j)
                    if prev is not None:
                        emit_pv(*prev)
                    prev = (j, pT)
                emit_pv(*prev)

                # softmax normalization for both heads of this region
                lt_sb = cp.tile([128, T // 128], F32, tag="lt", name=f"lt{p}_{n}")
                rt_sb = cp.tile([128, T // 128], F32, tag="rt", name=f"rt{p}_{n}")
                r_sb = cp.tile([1, T], F32, tag="r", name=f"r{p}_{n}")
                rb_sb = cp.tile([128, T], F32, tag="rb", name=f"rb{p}_{n}")
                for h, oT in ((2 * p, oTa), (2 * p + 1, oTb)):
                    hp = 64 * (h % 2)
                    nc.vector.tensor_copy(
                        y_sb[hp : hp + DH, p, c0r:c1r], oT[0:DH, :]
                    )
                    nc.vector.tensor_copy(
                        lrows[32 * h : 32 * h + 1, c0r:c1r], oT[DH : DH + 1, :]
                    )
                    nc.sync.dma_start(
                        lt_sb[32 * n : 32 * (n + 1), :],
                        lrows[32 * h : 32 * h + 1, c0r:c1r],
                    )
                    nc.vector.reciprocal(
                        rt_sb[32 * n : 32 * (n + 1), :],
                        lt_sb[32 * n : 32 * (n + 1), :],
                    )
                    nc.sync.dma_start(
                        r_sb[:, c0r:c1r], rt_sb[32 * n : 32 * (n + 1), :]
                    )
                    nc.gpsimd.partition_broadcast(
                        rb_sb[:, c0r:c1r], r_sb[:, c0r:c1r]
                    )
                    nc.vector.tensor_mul(
                        y_sb[hp : hp + DH, p, c0r:c1r],
                        y_sb[hp : hp + DH, p, c0r:c1r],
                        rb_sb[hp : hp + DH, c0r:c1r],
                    )

            # ---- main fused loop ----
            # minimal prefix before the first exp can flow: pair-0 halves of
            # q/k chunk 0 plus all of v chunk 0; everything else is filler
            proj_qk(0, wq_sb, qsb, 0)()
            proj_qk(0, wk_sb, ksb, 0)()
            proj_v(0, 0)()
            proj_v(0, 1)()

            fillers = deque()
            fillers.append(proj_qk(0, wq_sb, qsb, 1))
            fillers.append(proj_qk(0, wk_sb, ksb, 1))
            p3_left = deque()
            for n in range(NCH):
                if n < NCH - 1:
                    fillers.append(proj_qk(n + 1, wq_sb, qsb, 0))
                    fillers.append(proj_qk(n + 1, wk_sb, ksb, 0))
                    fillers.append(proj_qk(n + 1, wq_sb, qsb, 1))
                    fillers.append(proj_qk(n + 1, wk_sb, ksb, 1))
                    fillers.append(proj_v(n + 1, 0))
                    fillers.append(proj_v(n + 1, 1))
                if n == 1:

                    def dma_x2():
                        dma_x(2, nc.sync, nc.scalar)

                    fillers.appendleft(dma_x2)
                if n == 2:

                    def dma_x3():
                        dma_x(3, nc.sync, nc.scalar)

                    fillers.appendleft(dma_x3)
                # p3 of the previous chunk drains after the projections so
                # the PE never waits on the (long-latency) norm chain
                fillers.extend(p3_left)
                p3_left.clear()
                attention(0, n, fillers, 1)
                attention(1, n, fillers, 1)
                while fillers:
                    fillers.popleft()()
                p3_left.extend(p3_closures(n))
            while p3_left:
                p3_left.popleft()()
    nc.compile()
    return nc


def make_in_maps(x, Wq, Wk, Wv, Wo):
    import ml_dtypes

    cnp = ml_dtypes.bfloat16
    r = np.arange(128)
    tri = (-30000.0 * (r[:, None] > r[None, :])).astype(cnp)  # [tk, tq]
    idn = np.eye(128, dtype=cnp)
    in_maps = []
    for c in range(8):
        b, g = c // 4, c % 4
        rows = slice(DG * g, DG * (g + 1))
        in_maps.append(
            {
                "xT": np.ascontiguousarray(
                    x[b].T.reshape(CT, 128, NCH, 512).transpose(2, 1, 0, 3)
                ).astype(cnp),
                "wq": np.ascontiguousarray(
                    Wq[rows].T.reshape(CT, 128, DG).transpose(1, 0, 2)
                ).astype(cnp),
                "wk": np.ascontiguousarray(
                    Wk[rows].T.reshape(CT, 128, DG).transpose(1, 0, 2)
                ).astype(cnp),
                "wv": np.ascontiguousarray(
                    Wv[rows].T.reshape(CT, 128, DG).transpose(1, 0, 2)
                ).astype(cnp),
                "wo": np.ascontiguousarray(
                    Wo[:, rows].T.reshape(2, 128, D).transpose(1, 0, 2)
                ).astype(cnp),
                "tri": tri,
                "idn": idn,
            }
        )
    return in_maps


def _run(x, Wq, Wk, Wv, Wo, trace=False):
    if "nc" not in _CACHE:
        _CACHE["nc"] = build()
    nc = _CACHE["nc"]
    in_maps = make_in_maps(x, Wq, Wk, Wv, Wo)
    res = run_bass_kernel_spmd(nc, in_maps, core_ids=list(range(8)), trace=trace)
    out = np.zeros((B, T, D), dtype=np.float32)
    for c in range(8):
        out[c // 4] += np.asarray(res.results[c]["out"], dtype=np.float32)
    return out, res


def kernel(x, Wq, Wk, Wv, Wo):
    out, _ = _run(
        np.asarray(x, dtype=np.float32),
        np.asarray(Wq, dtype=np.float32),
        np.asarray(Wk, dtype=np.float32),
        np.asarray(Wv, dtype=np.float32),
        np.asarray(Wo, dtype=np.float32),
    )
    return out


# revision 13
# speedup vs baseline: 1.1867x; 1.0497x over previous
"""Causal self-attention (B=2, T=2048, D=1024, H=16, Dh=64) on 8 TRN2 cores.

Sharding: core c -> batch b = c//4 (data parallel), head group g = c%4
(tensor parallel, 4 heads = 256 dims). Each core computes a full-shape
[T, D] bf16 partial of the output projection for its (b, g); the host
sums the 4 head-group partials per batch in f32.

Fully fused chunk-major pipeline (chunk n = 512 query cols = region n):
for each chunk: q/k/v projections of chunk n+1 and out-proj of chunk
n-1 are emitted as PE "filler" work interleaved into the attention
j-stream of chunk n, so the ScalarE exp stream (the phase-2 bottleneck,
~75us) hides under PE work and the PE never idles waiting on exp.
Causal diag masking is done on the PE itself (identity-stationary
matmul accumulating a lower-triangular -3e4 constant into S.T before
exp) instead of VectorE multiplies, removing the Scalar->Vector->PE
dependency hop. All PSUM rotates through one shared [128,1024] slot
tag (2 bufs) + the O.T accumulators (2x2 bufs), exactly 8 banks.
"""

import numpy as np
from collections import deque
from contextlib import ExitStack

import concourse.bass as bass
import concourse.tile as tile
from concourse import bacc, mybir
from concourse.bass_utils import run_bass_kernel_spmd

F32 = mybir.dt.float32
BF16 = mybir.dt.bfloat16
CDT = BF16

B, T, D = 2, 2048, 1024
H_TOT, DH = 16, 64
HL = 4                # local heads per core
DG = HL * DH          # 256 local head dims
NT = T // 128         # 16 t-tiles
NCH = T // 512        # 4 t-chunks
CT = D // 128         # 8 c-tiles

_CACHE = {}


def build():
    nc = bacc.Bacc("TRN2", target_bir_lowering=False, debug=False, num_devices=8)
    xT_d = nc.dram_tensor("xT", [NCH, 128, CT, 512], CDT, kind="ExternalInput").ap()
    wq_d = nc.dram_tensor("wq", [128, CT, DG], CDT, kind="ExternalInput").ap()
    wk_d = nc.dram_tensor("wk", [128, CT, DG], CDT, kind="ExternalInput").ap()
    wv_d = nc.dram_tensor("wv", [128, CT, DG], CDT, kind="ExternalInput").ap()
    wo_d = nc.dram_tensor("wo", [128, 2, D], CDT, kind="ExternalInput").ap()
    tri_d = nc.dram_tensor("tri", [128, 128], CDT, kind="ExternalInput").ap()
    idn_d = nc.dram_tensor("idn", [128, 128], CDT, kind="ExternalInput").ap()
    out_d = nc.dram_tensor("out", [T, D], CDT, kind="ExternalOutput").ap()

    with tile.TileContext(nc) as tc:
        with ExitStack() as ctx:
            cons = ctx.enter_context(tc.tile_pool(name="cons", bufs=1))
            xp = ctx.enter_context(tc.tile_pool(name="xp", bufs=2))
            cp = ctx.enter_context(tc.tile_pool(name="cp", bufs=3))
            pp = ctx.enter_context(tc.tile_pool(name="pp", bufs=4))
            outp = ctx.enter_context(tc.tile_pool(name="outp", bufs=4))
            ps = ctx.enter_context(tc.tile_pool(name="ps", bufs=2, space="PSUM"))
            pj = ctx.enter_context(tc.tile_pool(name="pj", bufs=1, space="PSUM"))
            opool = ctx.enter_context(
                tc.tile_pool(name="opool", bufs=1, space="PSUM")
            )

            wq_sb = cons.tile([128, CT, DG], CDT)
            wk_sb = cons.tile([128, CT, DG], CDT)
            wv_sb = cons.tile([128, CT, DG], CDT)
            wo_sb = cons.tile([128, 2, D], CDT)
            tri_sb = cons.tile([128, 128], CDT)
            idn_sb = cons.tile([128, 128], CDT)

            # warm the ACT exp table during the initial DMA wait
            warm = cons.tile([1, 8], F32)
            nc.scalar.activation(
                warm[:], warm[:], mybir.ActivationFunctionType.Exp
            )

            qsb = cons.tile([128, 2, T], CDT)
            ksb = cons.tile([128, 2, T], CDT)
            lrows = cons.tile([128, T], F32)
            v_sb = cons.tile([128, NT, HL, DH + 1], CDT)
            nc.vector.memset(v_sb[:, :, :, DH], 1.0)
            y_sb = cons.tile([128, 2, T], CDT)

            # ---- input DMA, priority order: wq + x0 first, wk next (all
            # needed within ~5us), then prefetch x1 / wv / consts / wo ----
            x_tiles = [None] * NCH

            def dma_x(n, engs, nsplit=2):
                x_tiles[n] = xp.tile([128, CT, 512], CDT, tag="x", name=f"x{n}")
                step = CT // nsplit
                for s in range(nsplit):
                    engs[s % len(engs)].dma_start(
                        x_tiles[n][:, s * step : (s + 1) * step, :],
                        xT_d[n, :, s * step : (s + 1) * step, :],
                    )

            # max parallelism on the critical first loads: wq + wk + x0
            # split into many queue streams so the first matmul starts early
            for s in range(4):
                nc.sync.dma_start(
                    wq_sb[:, 2 * s : 2 * s + 2, :], wq_d[:, 2 * s : 2 * s + 2, :]
                )
            dma_x(0, [nc.scalar, nc.gpsimd], nsplit=8)
            for s in range(4):
                nc.sync.dma_start(
                    wk_sb[:, 2 * s : 2 * s + 2, :], wk_d[:, 2 * s : 2 * s + 2, :]
                )
            nc.sync.dma_start(tri_sb[:], tri_d[:])
            nc.sync.dma_start(idn_sb[:], idn_d[:])
            nc.scalar.dma_start(wv_sb[:], wv_d[:])
            dma_x(1, [nc.sync, nc.gpsimd])
            nc.gpsimd.dma_start(wo_sb[:], wo_d[:])

            # ---- PE filler closures: projections of chunk n (q, k, v) and
            # out-projection of earlier chunks, drained into the attention
            # j-stream so the PE stays busy while ScalarE chews exp ----
            def proj_qk(n, w_sb, dst, j2):
                # q/k projection of chunk n, pair half j2, as two filler
                # closures of 4 accumulating matmuls each (cast in the 2nd)
                slot = {}

                def half(h):
                    def run():
                        if "t" not in slot:
                            slot["t"] = pj.tile(
                                [128, 1024], F32, tag="pj", name=f"qk{n}_{j2}"
                            )
                        for ct in range(4 * h, 4 * h + 4):
                            nc.tensor.matmul(
                                slot["t"][:, 0:512],
                                w_sb[:, ct, 128 * j2 : 128 * (j2 + 1)],
                                x_tiles[n][:, ct, :],
                                start=(ct == 0),
                                stop=(ct == CT - 1),
                                skip_group_check=True,
                            )
                        if h == 1:
                            nc.vector.tensor_copy(
                                dst[:, j2, 512 * n : 512 * (n + 1)],
                                slot["t"][:, 0:512],
                            )

                    return run

                return [half(0), half(1)]

            def proj_v(n, h2):
                # v projection of chunk n for t-subtiles {2*h2, 2*h2+1}
                def run():
                    slot = pj.tile([128, 1024], F32, tag="pj", name=f"v{n}_{h2}")
                    for i in (2 * h2, 2 * h2 + 1):
                        for ct in range(CT):
                            nc.tensor.matmul(
                                slot[:, 256 * (i % 2) : 256 * (i % 2 + 1)],
                                x_tiles[n][:, ct, 128 * i : 128 * (i + 1)],
                                wv_sb[:, ct, :],
                                start=(ct == 0),
                                stop=(ct == CT - 1),
                                skip_group_check=True,
                            )
                    ti = 4 * n + 2 * h2
                    nc.vector.tensor_copy(
                        v_sb[:, ti : ti + 2, :, 0:DH],
                        slot[:, 0:512].rearrange(
                            "p (i h d) -> p i h d", i=2, h=HL
                        ),
                    )

                return run

            def p3_closures(n):
                # out[t,:] for t-tiles of chunk n; both oc halves in one slot
                cls = []
                for i in range(4 * n, 4 * n + 4):

                    def run(i=i):
                        po = pj.tile([128, 1024], F32, tag="pj", name=f"po{i}")
                        for oc in range(2):
                            for g2 in range(2):
                                nc.tensor.matmul(
                                    po[:, 512 * oc : 512 * (oc + 1)],
                                    y_sb[:, g2, 128 * i : 128 * (i + 1)],
                                    wo_sb[:, g2, 512 * oc : 512 * (oc + 1)],
                                    start=(g2 == 0),
                                    stop=(g2 == 1),
                                    skip_group_check=True,
                                )
                        o_sb = outp.tile([128, 1024], CDT, tag="o")
                        nc.vector.tensor_copy(o_sb[:], po[:])
                        eng = (nc.sync, nc.scalar)[i % 2]
                        eng.dma_start(
                            out_d[128 * i : 128 * (i + 1), :], o_sb[:]
                        )

                    cls.append(run)
                return cls

            # ---- attention for (pair p, region n) with filler draining ----
            def attention(p, n, fillers, pops):
                c0r, c1r = 512 * n, 512 * (n + 1)
                jlast = 4 * n + 3
                oTa = opool.tile([DH + 1, 512], F32, tag="oTa", name=f"oTa{p}_{n}")
                oTb = opool.tile([DH + 1, 512], F32, tag="oTb", name=f"oTb{p}_{n}")

                def emit_st(j):
                    c0 = max(c0r, 128 * j)
                    w = c1r - c0
                    masked = j >= 4 * n
                    sT = ps.tile([128, 1024], F32, tag="ps", name=f"sT{p}_{n}_{j}")
                    nc.tensor.matmul(
                        sT[:, 0:w],
                        ksb[0:DH, p, 128 * j : 128 * (j + 1)],
                        qsb[0:DH, p, c0:c1r],
                        start=True,
                        stop=not masked,
                        skip_group_check=True,
                    )
                    nc.tensor.matmul(
                        sT[:, 512 : 512 + w],
                        ksb[DH:128, p, 128 * j : 128 * (j + 1)],
                        qsb[DH:128, p, c0:c1r],
                        start=True,
                        stop=not masked,
                        skip_group_check=True,
                    )
                    if masked:  # diag block at rel cols [0,128): add -3e4 above
                        nc.tensor.matmul(
                            sT[:, 0:128], idn_sb[:], tri_sb[:],
                            start=False, stop=True, skip_group_check=True,
                        )
                        nc.tensor.matmul(
                            sT[:, 512:640], idn_sb[:], tri_sb[:],
                            start=False, stop=True, skip_group_check=True,
                        )
                    pT = pp.tile([128, 1024], CDT, tag="pT", name=f"pT{p}_{n}_{j}")
                    nc.scalar.activation(
                        pT[:, 0 : 512 + w],
                        sT[:, 0 : 512 + w],
                        mybir.ActivationFunctionType.Exp,
                        scale=0.125,
                    )
                    return pT

                def emit_pv(j, pT):
                    c0 = max(c0r, 128 * j)
                    w = c1r - c0
                    nc.tensor.matmul(
                        oTa[:, c0 - c0r :],
                        v_sb[:, j, 2 * p, :],
                        pT[:, 0:w],
                        start=(j == 0),
                        stop=(j == jlast),
                        skip_group_check=True,
                    )
                    nc.tensor.matmul(
                        oTb[:, c0 - c0r :],
                        v_sb[:, j, 2 * p + 1, :],
                        pT[:, 512 : 512 + w],
                        start=(j == 0),
                        stop=(j == jlast),
                        skip_group_check=True,
                    )

                prev = None
                for j in range(jlast + 1):
                    for _ in range(pops):
                        if fillers:
                            fillers.popleft()()
                    pT = emit_st(j)
                    if prev is not None:
                        emit_pv(*prev)
                    prev = (j, pT)
                emit_pv(*prev)

                # softmax normalization for both heads of this region
                lt_sb = cp.tile([128, T // 128], F32, tag="lt", name=f"lt{p}_{n}")
                rt_sb = cp.tile([128, T // 128], F32, tag="rt", name=f"rt{p}_{n}")
                r_sb = cp.tile([1, T], F32, tag="r", name=f"r{p}_{n}")
                rb_sb = cp.tile([128, T], F32, tag="rb", name=f"rb{p}_{n}")
                for h, oT in ((2 * p, oTa), (2 * p + 1, oTb)):
                    hp = 64 * (h % 2)
                    nc.vector.tensor_copy(
                        y_sb[hp : hp + DH, p, c0r:c1r], oT[0:DH, :]
                    )
                    nc.vector.tensor_copy(
                        lrows[32 * h : 32 * h + 1, c0r:c1r], oT[DH : DH + 1, :]
                    )
                    nc.sync.dma_start(
                        lt_sb[32 * n : 32 * (n + 1), :],
                        lrows[32 * h : 32 * h + 1, c0r:c1r],
                    )
                    nc.vector.reciprocal(
                        rt_sb[32 * n : 32 * (n + 1), :],
                        lt_sb[32 * n : 32 * (n + 1), :],
                    )
                    nc.sync.dma_start(
                        r_sb[:, c0r:c1r], rt_sb[32 * n : 32 * (n + 1), :]
                    )
                    nc.gpsimd.partition_broadcast(
                        rb_sb[:, c0r:c1r], r_sb[:, c0r:c1r]
                    )
                    nc.vector.tensor_mul(
                        y_sb[hp : hp + DH, p, c0r:c1r],
                        y_sb[hp : hp + DH, p, c0r:c1r],
                        rb_sb[hp : hp + DH, c0r:c1r],
                    )

            # ---- main fused loop ----
            # minimal prefix before the first exp can flow: pair-0 halves of
            # q/k chunk 0 + first half of v chunk 0; everything else fills
            for c in proj_qk(0, wq_sb, qsb, 0):
                c()
            for c in proj_qk(0, wk_sb, ksb, 0):
                c()
            proj_v(0, 0)()

            # fill_a drains in attention(pair 0, n): projections, x prefetch.
            # fill_b drains in attention(pair 1, n): p3 of chunk n-1, which
            # must never pop early (it waits on the long-latency norm chain).
            fill_a = deque()
            fill_b = deque()
            fill_a.append(proj_v(0, 1))
            fill_a.extend(proj_qk(0, wq_sb, qsb, 1))
            fill_a.extend(proj_qk(0, wk_sb, ksb, 1))
            for n in range(NCH):
                if n < NCH - 1:
                    fill_a.extend(proj_qk(n + 1, wq_sb, qsb, 0))
                    fill_a.extend(proj_qk(n + 1, wk_sb, ksb, 0))
                    fill_a.extend(proj_qk(n + 1, wq_sb, qsb, 1))
                    fill_a.extend(proj_qk(n + 1, wk_sb, ksb, 1))
                    fill_a.append(proj_v(n + 1, 0))
                    fill_a.append(proj_v(n + 1, 1))
                if n == 1:

                    def dma_x2():
                        dma_x(2, [nc.sync, nc.scalar])

                    fill_a.appendleft(dma_x2)
                if n == 2:

                    def dma_x3():
                        dma_x(3, [nc.sync, nc.scalar])

                    fill_a.appendleft(dma_x3)
                attention(0, n, fill_a, max(1, -(-len(fill_a) // (4 * n + 4))))
                attention(1, n, fill_b, max(1, -(-len(fill_b) // (4 * n + 4))))
                while fill_a:
                    fill_a.popleft()()
                while fill_b:
                    fill_b.popleft()()
                fill_b.extend(p3_closures(n))
            while fill_b:
                fill_b.popleft()()
    nc.compile()
    return nc


def make_in_maps(x, Wq, Wk, Wv, Wo):
    import ml_dtypes

    cnp = ml_dtypes.bfloat16
    r = np.arange(128)
    tri = (-30000.0 * (r[:, None] > r[None, :])).astype(cnp)  # [tk, tq]
    idn = np.eye(128, dtype=cnp)
    in_maps = []
    for c in range(8):
        b, g = c // 4, c % 4
        rows = slice(DG * g, DG * (g + 1))
        in_maps.append(
            {
                "xT": np.ascontiguousarray(
                    x[b].T.reshape(CT, 128, NCH, 512).transpose(2, 1, 0, 3)
                ).astype(cnp),
                "wq": np.ascontiguousarray(
                    Wq[rows].T.reshape(CT, 128, DG).transpose(1, 0, 2)
                ).astype(cnp),
                "wk": np.ascontiguousarray(
                    Wk[rows].T.reshape(CT, 128, DG).transpose(1, 0, 2)
                ).astype(cnp),
                "wv": np.ascontiguousarray(
                    Wv[rows].T.reshape(CT, 128, DG).transpose(1, 0, 2)
                ).astype(cnp),
                "wo": np.ascontiguousarray(
                    Wo[:, rows].T.reshape(2, 128, D).transpose(1, 0, 2)
                ).astype(cnp),
                "tri": tri,
                "idn": idn,
            }
        )
    return in_maps


def _run(x, Wq, Wk, Wv, Wo, trace=False):
    if "nc" not in _CACHE:
        _CACHE["nc"] = build()
    nc = _CACHE["nc"]
    in_maps = make_in_maps(x, Wq, Wk, Wv, Wo)
    res = run_bass_kernel_spmd(nc, in_maps, core_ids=list(range(8)), trace=trace)
    out = np.zeros((B, T, D), dtype=np.float32)
    for c in range(8):
        out[c // 4] += np.asarray(res.results[c]["out"], dtype=np.float32)
    return out, res


def kernel(x, Wq, Wk, Wv, Wo):
    out, _ = _run(
        np.asarray(x, dtype=np.float32),
        np.asarray(Wq, dtype=np.float32),
        np.asarray(Wk, dtype=np.float32),
        np.asarray(Wv, dtype=np.float32),
        np.asarray(Wo, dtype=np.float32),
    )
    return out
